# revision 24
# baseline (speedup 1.0000x reference)
"""Multi-head embedding lookup on 8 NeuronCores.

Sharding: head h -> core h. Each core owns one head's 100000x64 f32 table
shard and produces rows for all B*L = 65536 tokens of that head.

Per-core algorithm (int16-indexed Ant SWDGE ucode ops, <=1024 descriptors
per instruction -- the SWDGE ring limit measured on HW):
  - Table shard split into TQ=4 quartiles of 25000 rows (int16-addressable).
  - Tokens of each quartile, in token order, balanced-split into NCH_q
    chunks of <=1024.  Chunk j's token positions cluster around
    NTOK*j/NCH_q, so a static 32768-row output window per chunk keeps
    scatter positions int16-addressable (bases are computed from the
    actual inputs and asserted).
  - Per chunk: dma_gather (queue 0) table window rows -> SBUF wrap layout;
    dma_scatter_add (queue 1) SBUF rows -> out[token position] (+= onto the
    pre-zeroed output).
  - Chunks are padded to 1024 with -1 (skipped by the ucode); true counts
    ride in an int32 side tensor -> Pool registers.
"""

import contextlib
import os
import sys

if "/opt/trn_rl_repo" not in sys.path:
    sys.path.insert(0, "/opt/trn_rl_repo")

_SP = os.environ.get("KSP", "1") == "1"      # single_packet for gather/scatter
_SORT = os.environ.get("KSORT", "1") == "1"  # sort chunk entries by gather idx
_GQ = int(os.environ.get("KGQ", "1"))        # gather queues (round-robin)
_NQ = int(os.environ.get("KNQ", "2"))        # total swdge queues

import numpy as np

import concourse.bacc as bacc
import concourse.bass as bass
import concourse.mybir as mybir
from concourse.bass_utils import run_bass_kernel_spmd
from concourse.library_config import mlp

H = 8          # heads == cores
NH = 100000    # table rows per head
D = 64         # embedding dim
B, L = 16, 4096
NTOK = B * L   # tokens per head = 65536
TQ, QSZ = 4, 25000   # table quartiles
CAP = int(os.environ.get("KCAP", "1024"))  # max descriptors per SWDGE instruction
WIN = 32768          # scatter window rows (int16 reach)
NSLOT = int(os.environ.get("KNSLOT", "6"))  # SBUF data ring slots

_CACHE: dict = {}


def _core_quartiles(idx: np.ndarray):
    """idx: [NTOK] int64 -> per quartile (positions int64, local gather idx int64)."""
    q_of = idx // QSZ
    out = []
    for q in range(TQ):
        pos = np.nonzero(q_of == q)[0]
        out.append((pos, idx[pos] - q * QSZ))
    return out


def _plan(all_quart):
    """all_quart: [H][TQ](pos, gloc).  Returns static plan:
    chunks = list of (q, base) in issue order; nch[q]."""
    nch = []
    for q in range(TQ):
        nch.append(max(-(-len(all_quart[h][q][0]) // CAP) for h in range(H)))
    chunks = []
    for q in range(TQ):
        for j in range(nch[q]):
            base = NTOK
            lo_needed = 0
            for h in range(H):
                pos = all_quart[h][q][0]
                part = np.array_split(pos, nch[q])[j]
                assert len(part) > 0
                base = min(base, int(part[0]))
                lo_needed = max(lo_needed, int(part[-1]))
            assert lo_needed - base < WIN, (q, j, base, lo_needed)
            base = min(base, NTOK - WIN)  # keep window inside the tensor
            chunks.append((q, base))
    return chunks, nch


def _wrap16(vals: np.ndarray, pad: int) -> np.ndarray:
    a = np.full(pad, -1, dtype=np.int16)
    a[: len(vals)] = vals
    return a.reshape(pad // 16, 16).T  # [16, pad//16]


def _prep_core(quart, chunks, nch):
    ncols = len(chunks) * (CAP // 16)
    gidx_all = np.full((16, ncols), -1, dtype=np.int16)
    sidx_all = np.full((16, ncols), -1, dtype=np.int16)
    counts = np.zeros((1, len(chunks)), dtype=np.int32)
    # chunk order in `chunks` is q-major, j-minor — mirror it
    ci = 0
    for q in range(TQ):
        pos, gloc = quart[q]
        parts = np.array_split(np.arange(len(pos)), nch[q])
        for j in range(nch[q]):
            sel = parts[j]
            base = chunks[ci][1]
            g = gloc[sel]
            s = pos[sel] - base
            if _SORT:
                order = np.argsort(g, kind="stable")
                g = g[order]
                s = s[order]
            assert s.min() >= 0 and s.max() < WIN
            w = CAP // 16
            gidx_all[:, ci * w : (ci + 1) * w] = _wrap16(g.astype(np.int16), CAP)
            sidx_all[:, ci * w : (ci + 1) * w] = _wrap16(s.astype(np.int16), CAP)
            counts[0, ci] = len(sel)
            ci += 1
    gidx_all = np.ascontiguousarray(np.tile(gidx_all, (8, 1)))
    sidx_all = np.ascontiguousarray(np.tile(sidx_all, (8, 1)))
    return gidx_all, sidx_all, counts


def _build_nc(chunks, reps: int = 1, mode: str = "full") -> bass.Bass:
    NCH = len(chunks)
    ncols = NCH * (CAP // 16)
    nc = bacc.Bacc("TRN2", num_swdge_queues=_NQ)
    table = nc.dram_tensor("table", [NH, D], mybir.dt.float32, kind="ExternalInput")
    gidx = nc.dram_tensor("gidx", [128, ncols], mybir.dt.int16, kind="ExternalInput")
    sidx = nc.dram_tensor("sidx", [128, ncols], mybir.dt.int16, kind="ExternalInput")
    counts = nc.dram_tensor("counts", [1, NCH], mybir.dt.int32, kind="ExternalInput")
    out = nc.dram_tensor("out", [NTOK, D], mybir.dt.float32, kind="ExternalOutput")

    with contextlib.ExitStack() as ctx:
        gidx_t = ctx.enter_context(nc.sbuf_tensor("gidx_t", [128, ncols], mybir.dt.int16))
        sidx_t = ctx.enter_context(nc.sbuf_tensor("sidx_t", [128, ncols], mybir.dt.int16))
        counts_t = ctx.enter_context(nc.sbuf_tensor("counts_t", [1, NCH], mybir.dt.int32))
        data = [
            ctx.enter_context(
                nc.sbuf_tensor(f"data{s}", [128, CAP // 128, D], mybir.dt.float32)
            )
            for s in range(NSLOT)
        ]
        in_sems = [ctx.enter_context(nc.semaphore(f"in_sem{i}")) for i in range(3)]
        g_sems = [ctx.enter_context(nc.semaphore(f"g_sem{s}")) for s in range(NSLOT)]
        st_sems = [ctx.enter_context(nc.semaphore(f"st_sem{s}")) for s in range(NSLOT)]
        block = ctx.enter_context(nc.Block())

        @block.sync
        def _(sp):
            sp.dma_start(out=gidx_t[:], in_=gidx[:]).then_inc(in_sems[0], 16)
            sp.dma_start(out=sidx_t[:], in_=sidx[:]).then_inc(in_sems[1], 16)
            sp.dma_start(out=counts_t[:], in_=counts[:]).then_inc(in_sems[2], 16)

        @block.gpsimd
        def _(gp):
            gp.load_library(mlp)
            for i in range(3):
                gp.wait_ge(in_sems[i], 16)

            with contextlib.ExitStack() as rctx:
                cnt = [
                    rctx.enter_context(gp.register(f"cnt{s}")) for s in range(NSLOT)
                ]
                w = CAP // 16

                TOT = NCH * reps

                def issue_gather(t):
                    s, r = t % NSLOT, t // NSLOT
                    k = t % NCH
                    q = chunks[k][0]
                    if r > 0:
                        gp.wait_ge(st_sems[s], 16 * r)
                    gp.reg_load(cnt[s], counts_t[0:1, k : k + 1])
                    gp.dma_gather(
                        out_ap=data[s][:],
                        in_ap=table[q * QSZ : (q + 1) * QSZ, :],
                        idxs_ap=gidx_t[:, k * w : (k + 1) * w],
                        num_idxs=CAP,
                        num_idxs_reg=cnt[s],
                        elem_size=D,
                        queue_num=t % _GQ,
                        single_packet=_SP,
                    ).then_inc(g_sems[s], 16)

                def issue_scatter(t):
                    s, r = t % NSLOT, t // NSLOT
                    k = t % NCH
                    base = chunks[k][1]
                    gp.wait_ge(g_sems[s], 16 * (r + 1))
                    gp.dma_scatter_add(
                        out_ap=out[base : base + WIN, :],
                        in_ap=data[s][:],
                        idxs_ap=sidx_t[:, k * w : (k + 1) * w],
                        num_idxs=CAP,
                        num_idxs_reg=cnt[s],
                        elem_size=D,
                        queue_num=_NQ - 1,
                        single_packet=_SP,
                    ).then_inc(st_sems[s], 16)

                if mode == "full":
                    for t in range(TOT):
                        issue_gather(t)
                        if t >= 1:
                            issue_scatter(t - 1)
                    issue_scatter(TOT - 1)
                    for s in range(NSLOT):
                        gp.wait_ge(st_sems[s], 16 * len(range(s, TOT, NSLOT)))
                elif mode == "gather":
                    # timing probe: gathers only, no slot backpressure needed
                    for t in range(TOT):
                        s, k = t % NSLOT, t % NCH
                        q = chunks[k][0]
                        gp.reg_load(cnt[s], counts_t[0:1, k : k + 1])
                        gp.dma_gather(
                            out_ap=data[s][:],
                            in_ap=table[q * QSZ : (q + 1) * QSZ, :],
                            idxs_ap=gidx_t[:, k * w : (k + 1) * w],
                            num_idxs=CAP,
                            num_idxs_reg=cnt[s],
                            elem_size=D,
                            queue_num=0,
                        ).then_inc(g_sems[s], 16)
                    for s in range(NSLOT):
                        gp.wait_ge(g_sems[s], 16 * len(range(s, TOT, NSLOT)))
                elif mode == "scatter":
                    # timing probe: scatter garbage SBUF repeatedly
                    for t in range(TOT):
                        s, k = t % NSLOT, t % NCH
                        base = chunks[k][1]
                        gp.reg_load(cnt[s], counts_t[0:1, k : k + 1])
                        gp.dma_scatter_add(
                            out_ap=out[base : base + WIN, :],
                            in_ap=data[s][:],
                            idxs_ap=sidx_t[:, k * w : (k + 1) * w],
                            num_idxs=CAP,
                            num_idxs_reg=cnt[s],
                            elem_size=D,
                            queue_num=1,
                        ).then_inc(st_sems[s], 16)
                    for s in range(NSLOT):
                        gp.wait_ge(st_sems[s], 16 * len(range(s, TOT, NSLOT)))
                elif mode == "regload":
                    # timing probe: just the reg_loads
                    for t in range(TOT):
                        s, k = t % NSLOT, t % NCH
                        gp.reg_load(cnt[s], counts_t[0:1, k : k + 1])
                    gp.wait_ge(in_sems[0], 16)
                else:
                    raise ValueError(mode)

    nc.compile()
    return nc


def _get_nc(chunks):
    key = ("nc", tuple(chunks))
    if key not in _CACHE:
        _CACHE[key] = _build_nc(chunks)
    return _CACHE[key]


def kernel(input_ids: np.ndarray, table: np.ndarray, **_run_kw) -> np.ndarray:
    input_ids = np.asarray(input_ids)
    table = np.asarray(table, dtype=np.float32)

    all_quart = [
        _core_quartiles(input_ids[:, :, h].reshape(-1).astype(np.int64))
        for h in range(H)
    ]
    chunks, nch = _plan(all_quart)
    nc = _get_nc(chunks)

    in_maps = []
    for h in range(H):
        gidx_all, sidx_all, counts = _prep_core(all_quart[h], chunks, nch)
        tab_h = np.ascontiguousarray(table[h * NH : (h + 1) * NH])
        in_maps.append(
            {"table": tab_h, "gidx": gidx_all, "sidx": sidx_all, "counts": counts}
        )

    res = run_bass_kernel_spmd(nc, in_maps, list(range(H)), **_run_kw)
    outs = [
        np.asarray(res.results[h]["out"]).reshape(B, L, D) for h in range(H)
    ]
    full = np.stack(outs, axis=2)  # [B, L, H, D]
    if _run_kw:
        _CACHE["last_results"] = res
    return full
